# revision 18
# baseline (speedup 1.0000x reference)
"""ProTCL-style dense MLP over a [B=16, NL=5000] cross-join, on 8 TRN2 cores.

Math (reference):
    P_e = seq @ Wp.T; L_e = lab @ Wl.T
    h   = relu(P_e @ W1p.T [+broadcast] L_e @ W1l.T + b1)   # [B, NL, O]
    h   = relu(h @ W2.T + b2)                               # [B, NL, O]
    out = (h @ W3.T + b3)[..., 0]                           # [B, NL]

Strategy:
  - Shard the label axis across 8 cores (625 labels each); host gathers.
  - Host folds (W1l @ Wl) into one matrix Wfl so the device does a single
    matmul for hl = lab @ Wfl.T, and folds the whole (tiny, B=16) sequence
    path into hpb = seq @ (W1p @ Wp).T + b1 on the host.
  - Device works feature-major: features on partitions, (b, label) sample
    columns on the free axis. The dominant W2 matmul keeps W2 stationary
    and streams 20 chunks of 500 columns, accumulating over 8 k-tiles.
    The W3 matvecs are batched at the chunk tail so they don't break the
    W2 LDWEIGHTS prefetch pipeline.
  - bf16 operands with fp32 PSUM accumulation (measured ~5e-3 rel err).
"""

import numpy as np

B = 16
NL = 5000
D = 1024
O = 1024
N_CORES = 8
NLC = NL // N_CORES        # 625 labels per core
KT = O // 128              # 8 contraction tiles
MT = O // 128              # 8 output-feature tiles
CHUNK = 500                # sample columns per phase-2 chunk
COLS = B * NLC             # 10000 sample columns per core
NCHUNK = COLS // CHUNK     # 20

_CACHE = {}


def _patch_tile_drain():
    """This container's walrus codegen rejects >1 sync wait per instruction.
    Tile freely attaches one wait per producer domain. Two patches:
      1. _commit_instruction: before committing an instruction carrying N>1
         waits, emit N-1 single-wait NOPs on the same engine (engines execute
         their stream in order, so semantics are identical).
      2. The tail drain (emitted after the commit hook) gets the same
         treatment via _drain_and_barrier."""
    import concourse.mybir as mybir
    import concourse.tile as tile
    from concourse.vector_clock import ScopedClock

    if getattr(tile.TileContext, "_drain_patch_applied", False):
        return

    _orig_commit = tile.TileContext._commit_instruction

    def _commit_instruction(self, inst, lazy_reg_writes: bool = True):
        si = getattr(inst, "sync_info", None)
        if si is not None and len(si.on_wait) > 1:
            waits = list(si.on_wait)
            for w in waits[:-1]:
                nop = mybir.InstNoOp(
                    name=self.nc.get_next_instruction_name(),
                    engine=inst.engine,
                    sync_info=mybir.SyncInfo(on_wait=[w], on_update=[]),
                    bass_nofuse=True,
                )
                self._add_instruction(nop)
            inst.sync_info = mybir.SyncInfo(
                on_wait=[waits[-1]], on_update=list(si.on_update)
            )
        return _orig_commit(self, inst, lazy_reg_writes)

    tile.TileContext._commit_instruction = _commit_instruction

    def _drain_and_barrier(self, tick_clock, wait_clock):
        probe = self.nc.sync.nop(nofuse=True)
        wait_clock.add_sem_waits(
            probe.ins, ScopedClock({None: tick_clock.global_clock})
        )
        si = probe.ins.sync_info
        waits = list(si.on_wait) if si is not None else []
        if len(waits) > 1:
            probe.ins.sync_info = mybir.SyncInfo(
                on_wait=waits[:1], on_update=list(si.on_update)
            )
            for w in waits[1:]:
                extra = self.nc.sync.nop(nofuse=True)
                extra.ins.sync_info = mybir.SyncInfo(on_wait=[w], on_update=[])
        self.nc.sync.drain()
        self.nc.all_engine_barrier()
        assert self.sems is not None
        popped = self.nc._tile_sem_poison_stack.pop()
        assert popped is self._sem_poison
        self.nc.clear_and_free_semaphores(list(self.sems.allocated().values()))
        self.nc.all_engine_barrier()

    tile.TileContext._drain_and_barrier = _drain_and_barrier
    tile.TileContext._drain_patch_applied = True


def _build_nc():
    import concourse.bass as bass
    import concourse.mybir as mybir
    import concourse.tile as tile

    _patch_tile_drain()

    f32 = mybir.dt.float32
    bf16 = mybir.dt.bfloat16
    Relu = mybir.ActivationFunctionType.Relu
    add = mybir.AluOpType.add
    amax = mybir.AluOpType.max

    nc = bass.Bass("TRN2")

    labT_d = nc.declare_dram_parameter("labT", [128, KT, NLC], bf16, isOutput=False)
    wfl_d = nc.declare_dram_parameter("wflT", [128, KT, MT, 128], bf16, isOutput=False)
    w2_d = nc.declare_dram_parameter("w2T", [128, KT, MT, 128], bf16, isOutput=False)
    w3_d = nc.declare_dram_parameter("w3T", [128, MT], bf16, isOutput=False)
    hpb_d = nc.declare_dram_parameter("hpbT", [128, KT, B], f32, isOutput=False)
    b2_d = nc.declare_dram_parameter("b2T", [128, MT], f32, isOutput=False)
    out_d = nc.declare_dram_parameter("logits", [1, COLS], f32, isOutput=True)

    with tile.TileContext(nc) as tc:
        with (
            tc.tile_pool(name="const", bufs=1) as cpool,
            tc.tile_pool(name="h", bufs=4) as hpool,
            tc.tile_pool(name="y", bufs=10) as ypool,
            tc.tile_pool(name="ps", bufs=4, space="PSUM") as pspool,
            tc.tile_pool(name="pl", bufs=2, space="PSUM") as plpool,
        ):
            # Per-k tiles so dependencies (and DMAs) are fine-grained: the
            # first phase-1 matmul only waits for its own k-slice DMAs.
            labT = [cpool.tile([128, NLC], bf16, tag=f"labT{k}", name=f"labT{k}") for k in range(KT)]
            wfl = [cpool.tile([128, MT, 128], bf16, tag=f"wfl{k}", name=f"wfl{k}") for k in range(KT)]
            for k in range(KT):
                nc.sync.dma_start(labT[k][:], labT_d[:, k])
                nc.sync.dma_start(wfl[k][:], wfl_d[:, k])
            w2 = [cpool.tile([128, MT, 128], bf16, tag=f"w2{k}", name=f"w2{k}") for k in range(KT)]
            for k in range(KT):
                nc.sync.dma_start(w2[k][:], w2_d[:, k])
            w3 = cpool.tile([128, MT], bf16)
            nc.sync.dma_start(w3[:], w3_d[:])
            hpb = cpool.tile([128, KT, B], f32)
            nc.sync.dma_start(hpb[:], hpb_d[:])
            b2 = cpool.tile([128, MT], f32)
            nc.sync.dma_start(b2[:], b2_d[:])

            logits_sb = cpool.tile([1, COLS], f32)
            hlT = [cpool.tile([128, NLC], bf16, tag=f"hlT{k}", name=f"hlT{k}") for k in range(KT)]

            # ---- phase 0: PE warmup on junk data while input DMAs stream ----
            # Keeps the HAM clock-gate busy (reaches 2.4 GHz before real work)
            # and absorbs the initial DMA latency.
            warm = cpool.tile([128, 512], bf16)
            nc.gpsimd.memset(warm[:], 0.0)
            wps = pspool.tile([128, 512], f32, tag="ps", name="warm_ps")
            for _ in range(34):
                nc.tensor.matmul(
                    wps[:], lhsT=warm[:, :128], rhs=warm[:], start=True, stop=True
                )

            # ---- phase 1: hlT[m][l] = (Wfl @ labT)[m-tile] ----
            # k-outer over half the m-tiles at a time (4 PSUM banks), so the
            # first matmul starts as soon as the k=0 slices have landed.
            for n0, w in ((0, 512), (512, NLC - 512)):
                for half in (0, 4):
                    pss = [
                        pspool.tile([128, 512], f32, tag="ps", name=f"ps1_{n0}_{half}_{i}")
                        for i in range(4)
                    ]
                    for k in range(KT):
                        for mi in range(4):
                            m = half + mi
                            nc.tensor.matmul(
                                pss[mi][:, :w],
                                lhsT=wfl[k][:, m, :],
                                rhs=labT[k][:, n0 : n0 + w],
                                start=(k == 0),
                                stop=(k == KT - 1),
                            )
                    for mi in range(4):
                        m = half + mi
                        if m % 2 == 0:
                            nc.vector.tensor_copy(
                                hlT[m][:, n0 : n0 + w], pss[mi][:, :w]
                            )
                        else:
                            nc.scalar.copy(hlT[m][:, n0 : n0 + w], pss[mi][:, :w])

            # ---- phase 2: 20 chunks of 500 sample columns ----
            def emit_logits_chain(lg_prev, p0):
                # partition-sum rows {0,32,64,96} of the matvec PSUM into the
                # SBUF logits slice, then flush that chunk to DRAM. Deferred
                # to after the NEXT chunk's h-prep so DVE never blocks on PE.
                lslice = logits_sb[:, p0 : p0 + CHUNK]
                nc.vector.tensor_copy(lslice, lg_prev[0:1, :])
                for row in (32, 64, 96):
                    nc.vector.tensor_tensor(
                        lslice, lg_prev[row : row + 1, :], lslice, mybir.AluOpType.add
                    )
                nc.sync.dma_start(out_d[:, p0 : p0 + CHUNK], lslice)

            pending = None
            for ci in range(NCHUNK):
                c0 = ci * CHUNK
                segs = []
                b_first = c0 // NLC
                b_last = (c0 + CHUNK - 1) // NLC
                for b in range(b_first, b_last + 1):
                    lo = max(c0, b * NLC)
                    hi = min(c0 + CHUNK, (b + 1) * NLC)
                    segs.append((b, lo, hi))

                h_sb = hpool.tile([128, KT, CHUNK], bf16, tag="h", name=f"h_{ci}")
                for k in range(KT):
                    for b, lo, hi in segs:
                        nc.vector.tensor_scalar(
                            h_sb[:, k, lo - c0 : hi - c0],
                            hlT[k][:, lo - b * NLC : hi - b * NLC],
                            hpb[:, k, b : b + 1],
                            0.0,
                            add,
                            amax,
                        )
                if pending is not None:
                    emit_logits_chain(*pending)

                y_tiles = []
                for m in range(MT):
                    ps = pspool.tile([128, 512], f32, tag="ps")
                    for k in range(KT):
                        nc.tensor.matmul(
                            ps[:, :CHUNK],
                            lhsT=w2[k][:, m, :],
                            rhs=h_sb[:, k, :],
                            start=(k == 0),
                            stop=(k == KT - 1),
                        )
                    y_sb = ypool.tile([128, CHUNK], bf16, tag="y")
                    if m != MT - 1:
                        nc.scalar.activation(
                            y_sb[:], ps[:, :CHUNK], Relu, bias=b2[:, m : m + 1]
                        )
                    else:
                        nc.vector.tensor_scalar(
                            y_sb[:], ps[:, :CHUNK], b2[:, m : m + 1], 0.0, add, amax
                        )
                    y_tiles.append(y_sb)

                # W3 matvecs batched at the chunk tail (doesn't perturb the
                # W2 weight-prefetch pipeline). Col-tiled: 4 concurrent M=1
                # matmuls in distinct 32-column PE groups, 2 accumulating
                # rounds, then a 3-op partition sum.
                lg = plpool.tile([128, CHUNK], f32, tag="lg")
                for r in range(2):
                    for j in range(4):
                        m = r * 4 + j
                        nc.tensor.matmul(
                            lg[32 * j : 32 * j + 1, :],
                            lhsT=w3[:, m : m + 1],
                            rhs=y_tiles[m][:],
                            start=(r == 0),
                            stop=(r == 1),
                            tile_position=(0, 32 * j),
                        )
                pending = (lg, c0)

            emit_logits_chain(*pending)

    return nc


def kernel(
    sequence_embeddings,
    label_embeddings,
    Wp,
    Wl,
    W1,
    b1,
    W2,
    b2,
    W3,
    b3,
):
    import ml_dtypes
    from concourse.bass_utils import run_bass_kernel_spmd

    seq = np.asarray(sequence_embeddings, np.float32)
    lab = np.asarray(label_embeddings, np.float32)
    Wp = np.asarray(Wp, np.float32)
    Wl = np.asarray(Wl, np.float32)
    W1 = np.asarray(W1, np.float32)
    b1 = np.asarray(b1, np.float32)
    W2 = np.asarray(W2, np.float32)
    b2 = np.asarray(b2, np.float32)
    W3 = np.asarray(W3, np.float32)
    b3 = np.asarray(b3, np.float32)

    bf = ml_dtypes.bfloat16

    # Host-side algebraic folds (cheap: 2.1 GFLOP + 36 MFLOP).
    Wfl = W1[:, D:] @ Wl                       # [O, L_DIM]
    hpb = seq @ (W1[:, :D] @ Wp).T + b1        # [B, O]

    # lhsT layouts: arr[kp, k, m, mc] = M.T[k*128+kp, m*128+mc]
    wflT = np.ascontiguousarray(
        Wfl.T.reshape(KT, 128, MT, 128).transpose(1, 0, 2, 3)
    ).astype(bf)
    w2T = np.ascontiguousarray(
        W2.T.reshape(KT, 128, MT, 128).transpose(1, 0, 2, 3)
    ).astype(bf)
    w3T = np.ascontiguousarray(W3[0].reshape(MT, 128).T).astype(bf)
    hpbT = np.ascontiguousarray(hpb.T.reshape(KT, 128, B).transpose(1, 0, 2)).astype(
        np.float32
    )
    b2T = np.ascontiguousarray(b2.reshape(MT, 128).T).astype(np.float32)

    in_maps = []
    for c in range(N_CORES):
        shard = lab[c * NLC : (c + 1) * NLC]   # [NLC, L_DIM]
        labT = np.ascontiguousarray(
            shard.T.reshape(KT, 128, NLC).transpose(1, 0, 2)
        ).astype(bf)
        in_maps.append(
            {
                "labT": labT,
                "wflT": wflT,
                "w2T": w2T,
                "w3T": w3T,
                "hpbT": hpbT,
                "b2T": b2T,
            }
        )

    if "nc" not in _CACHE:
        _CACHE["nc"] = _build_nc()
    nc = _CACHE["nc"]

    res = run_bass_kernel_spmd(nc, in_maps, core_ids=list(range(N_CORES)))

    # results[c]["logits"]: [1, COLS] fp32 ordered (b, label-within-shard)
    parts = [res.results[c]["logits"].reshape(B, NLC) for c in range(N_CORES)]
    out = np.concatenate(parts, axis=1) + b3[0]
    return out.astype(np.float32)
